# revision 1
# baseline (speedup 1.0000x reference)
"""MHA kernel for Trainium2, 8-way sharded (batch x head-group).

Reference: out = softmax((q@Wq+bq)(k@Wk+bk)^T / sqrt(64)) (v@Wv+bv) @ Wo + bo
Shapes: q,k,v [2, 2048, 768]; 12 heads x 64 dim.

Sharding (Megatron column-parallel): core c in 0..7 -> batch b = c//4,
head group g = c%4 (3 heads = channel slice 192g:192(g+1)). Each core
computes its heads' projections, attention, and partial out-proj
(Wo rows for its heads). Host sums the 4 partials per batch (+bo).

Device layout notes:
- Host pre-transposes q/k/v to [768, 2048] (bf16) so projections can
  contract over the partition dim without on-device transposes.
- Scores are computed transposed: S^T[k, q] = Kh^T.T @ Qh^T, so exp
  output P^T[k, q] feeds PV as the moving operand with lhsT = [Vh | 1]
  (the appended ones column yields the softmax row-sums for free).
- Softmax skips max-subtraction: scores ~ N(0,1), no overflow in fp32.
- Normalization: r = 1/l broadcast across partitions via a K=1 matmul
  (ones[1,64].T @ r[1,q]), then DVE multiply during PSUM evacuation.
"""

import sys

if "/opt/trn_rl_repo" not in sys.path:
    sys.path.insert(0, "/opt/trn_rl_repo")

import numpy as np
import ml_dtypes

S = 2048
D = 768
DH = 64
HG = 3          # heads per core
CS = HG * DH    # 192 channel slice per core
NCORES = 8
ECH = D // 128  # 6 contraction chunks

_cached = {}


def _build_nc():
    import concourse.bass as bass
    from concourse import bacc
    import concourse.mybir as mybir
    import concourse.tile as tile

    f32 = mybir.dt.float32
    f32r = mybir.dt.float32r
    bf16 = mybir.dt.bfloat16
    FP = mybir.dt.float32  # on-chip compute dtype

    nc = bacc.Bacc(None, target_bir_lowering=False)

    xq_d = nc.dram_tensor("xqT", [D, S], bf16, kind="ExternalInput")
    xk_d = nc.dram_tensor("xkT", [D, S], bf16, kind="ExternalInput")
    xv_d = nc.dram_tensor("xvT", [D, S], bf16, kind="ExternalInput")
    wq_d = nc.dram_tensor("wq", [D, CS], bf16, kind="ExternalInput")
    wk_d = nc.dram_tensor("wk", [D, CS], bf16, kind="ExternalInput")
    wv_d = nc.dram_tensor("wv", [D, CS], bf16, kind="ExternalInput")
    wo_d = nc.dram_tensor("wo", [CS, D], f32r, kind="ExternalInput")
    bq_d = nc.dram_tensor("bq", [CS, 1], f32, kind="ExternalInput")
    bk_d = nc.dram_tensor("bk", [CS, 1], f32, kind="ExternalInput")
    bv_d = nc.dram_tensor("bv", [128, CS], f32, kind="ExternalInput")
    out_d = nc.dram_tensor("out", [S, D], f32, kind="ExternalOutput")

    Exp = mybir.ActivationFunctionType.Exp
    PSUM = bass.MemorySpace.PSUM

    with tile.TileContext(nc) as tc:
        with (
            tc.tile_pool(name="cst", bufs=1) as cst,
            tc.tile_pool(name="big", bufs=1) as bigp,
            tc.tile_pool(name="pt", bufs=3) as ptp,
            tc.tile_pool(name="rr", bufs=2) as rrp,
            tc.tile_pool(name="osb", bufs=2) as osbp,
            tc.tile_pool(name="psA", bufs=2, space=PSUM) as psA,
            tc.tile_pool(name="psB", bufs=2, space=PSUM) as psB,
        ):
            # ---- constants / weights ----
            wq_sb = cst.tile([128, ECH, CS], bf16, tag="wq")
            nc.sync.dma_start(wq_sb[:], xq_rearr(wq_d))
            wk_sb = cst.tile([128, ECH, CS], bf16, tag="wk")
            nc.sync.dma_start(wk_sb[:], xq_rearr(wk_d))
            wv_sb = cst.tile([128, ECH, CS], bf16, tag="wv")
            nc.sync.dma_start(wv_sb[:], xq_rearr(wv_d))

            wo_sb = []
            for h in range(HG):
                t = cst.tile([DH, D], f32r, tag=f"wo{h}")
                nc.sync.dma_start(t[:], wo_d[h * DH:(h + 1) * DH, :])
                wo_sb.append(t)

            bq_a = cst.tile([128, 1], f32, tag="bqa")
            nc.sync.dma_start(bq_a[:], bq_d[0:128, :])
            bq_b = cst.tile([DH, 1], f32, tag="bqb")
            nc.sync.dma_start(bq_b[:], bq_d[128:CS, :])
            bk_a = cst.tile([128, 1], f32, tag="bka")
            nc.sync.dma_start(bk_a[:], bk_d[0:128, :])
            bk_b = cst.tile([DH, 1], f32, tag="bkb")
            nc.sync.dma_start(bk_b[:], bk_d[128:CS, :])
            bv_sb = cst.tile([128, CS], f32, tag="bv")
            nc.sync.dma_start(bv_sb[:], bv_d[:])

            ones_f = cst.tile([1, DH], f32, tag="onesf")
            nc.vector.memset(ones_f[:], 1.0)
            ones_c = cst.tile([1, DH], f32r, tag="ones")
            nc.vector.tensor_copy(ones_c[:], ones_f[:])
            onecol_f = cst.tile([128, HG, 1], f32, tag="onecf")
            nc.vector.memset(onecol_f[:], 1.0)
            onecol_r = cst.tile([128, HG, 1], f32r, tag="onecr")
            nc.vector.tensor_copy(onecol_r[:], onecol_f[:])

            # ---- inputs (per-chunk DMAs so projections start early) ----
            xq_sb = bigp.tile([128, ECH, S], bf16, tag="xq")
            xk_sb = bigp.tile([128, ECH, S], bf16, tag="xk")
            xv_sb = bigp.tile([128, ECH, S], bf16, tag="xv")
            for e in range(ECH):
                nc.sync.dma_start(xq_sb[:, e, :], xq_d[e * 128:(e + 1) * 128, :])
                nc.sync.dma_start(xk_sb[:, e, :], xk_d[e * 128:(e + 1) * 128, :])
                nc.sync.dma_start(xv_sb[:, e, :], xv_d[e * 128:(e + 1) * 128, :])

            # ---- projections ----
            # Q/K transposed per-head layout: heads 0,1 stacked [128, S]; head2 [64, S]
            qhT_a = bigp.tile([128, S], f32r, tag="qa")
            qhT_b = bigp.tile([DH, S], f32r, tag="qb")
            khT_a = bigp.tile([128, S], f32r, tag="ka")
            khT_b = bigp.tile([DH, S], f32r, tag="kb")

            for x_sb, w_sb, b_a, b_b, o_a, o_b in (
                (xq_sb, wq_sb, bq_a, bq_b, qhT_a, qhT_b),
                (xk_sb, wk_sb, bk_a, bk_b, khT_a, khT_b),
            ):
                for mc0, mw, bias, dest in ((0, 128, b_a, o_a), (128, DH, b_b, o_b)):
                    for sc in range(4):
                        ps = psB.tile([mw, 512], f32, tag="acc")
                        for e in range(ECH):
                            nc.tensor.matmul(
                                ps[:],
                                w_sb[:, e, mc0:mc0 + mw],
                                x_sb[:, e, sc * 512:(sc + 1) * 512],
                                start=(e == 0),
                                stop=(e == ECH - 1),
                            )
                        nc.vector.tensor_scalar_add(
                            dest[:, sc * 512:(sc + 1) * 512], ps[:], bias[:]
                        )

            # V natural layout [s, 3, 65] with ones in column 64
            vh = bigp.tile([128, 16, HG, DH + 1], f32r, tag="vh")
            for sb in range(16):
                ps = psB.tile([128, CS], f32, tag="acc")
                for e in range(ECH):
                    nc.tensor.matmul(
                        ps[:],
                        xv_sb[:, e, sb * 128:(sb + 1) * 128],
                        wv_sb[:, e, :],
                        start=(e == 0),
                        stop=(e == ECH - 1),
                    )
                nc.vector.tensor_copy(vh[:, sb, :, DH:DH + 1], onecol_r[:])
                nc.vector.tensor_add(
                    vh[:, sb, :, 0:DH],
                    ps[:].rearrange("p (h d) -> p h d", h=HG),
                    bv_sb[:].rearrange("p (h d) -> p h d", h=HG),
                )

            # ---- attention ----
            ohT = []
            for h in range(HG):
                ohT.append(bigp.tile([DH, S], f32r, tag=f"oh{h}", name=f"oh{h}"))

            for h in range(HG):
                if h < 2:
                    qh = qhT_a[h * DH:(h + 1) * DH, :]
                    kh = khT_a[h * DH:(h + 1) * DH, :]
                else:
                    qh = qhT_b[:, :]
                    kh = khT_b[:, :]
                for qb in range(2):  # q blocks of 1024
                    q0 = qb * 1024
                    po = psB.tile([DH + 1, 1024], f32, tag="acc")
                    for kc in range(16):  # k chunks of 128
                        ps = psA.tile([128, 1024], f32, tag="sc")
                        for nh in range(2):
                            nc.tensor.matmul(
                                ps[:, nh * 512:(nh + 1) * 512],
                                kh[:, kc * 128:(kc + 1) * 128],
                                qh[:, q0 + nh * 512:q0 + (nh + 1) * 512],
                            )
                        pt = ptp.tile([128, 1024], f32r, tag="pt")
                        nc.scalar.activation(pt[:], ps[:], Exp, scale=0.125)
                        for nh in range(2):
                            nc.tensor.matmul(
                                po[:, nh * 512:(nh + 1) * 512],
                                vh[:, kc, h, :],
                                pt[:, nh * 512:(nh + 1) * 512],
                                start=(kc == 0),
                                stop=(kc == 15),
                            )
                    # normalize: r = 1/l ; R = ones^T @ r ; ohT = po * R
                    r_sb = rrp.tile([1, 1024], f32r, tag="r")
                    with nc.allow_low_precision(reason="softmax denom in f32r"):
                        nc.vector.reciprocal(r_sb[:], po[DH:DH + 1, :])
                    R_ps = psB.tile([DH, 1024], f32, tag="acc")
                    for nh in range(2):
                        nc.tensor.matmul(
                            R_ps[:, nh * 512:(nh + 1) * 512],
                            ones_c[:],
                            r_sb[:, nh * 512:(nh + 1) * 512],
                        )
                    R_sb = rrp.tile([DH, 1024], f32, tag="R")
                    nc.vector.tensor_copy(R_sb[:], R_ps[:])
                    nc.vector.tensor_mul(
                        ohT[h][:, q0:q0 + 1024], po[0:DH, :], R_sb[:]
                    )

            # ---- out projection (partial; host adds bo and reduces) ----
            for qblk in range(16):
                o_sb = osbp.tile([128, D], f32, tag="osb")
                for half in range(2):
                    ps = psB.tile([128, 384], f32, tag="acc")
                    for h in range(HG):
                        nc.tensor.matmul(
                            ps[:],
                            ohT[h][:, qblk * 128:(qblk + 1) * 128],
                            wo_sb[h][:, half * 384:(half + 1) * 384],
                            start=(h == 0),
                            stop=(h == HG - 1),
                        )
                    nc.vector.tensor_copy(o_sb[:, half * 384:(half + 1) * 384], ps[:])
                nc.sync.dma_start(out_d[qblk * 128:(qblk + 1) * 128, :], o_sb[:])

    nc.compile()
    return nc


def xq_rearr(d):
    # [C*128, N] dram -> [128, C, N] (chunk-major partition layout)
    return d[:].rearrange("(c p) n -> p c n", p=128)


def _get_nc():
    if "nc" not in _cached:
        _cached["nc"] = _build_nc()
    return _cached["nc"]


def kernel(q, k, v, Wq, bq, Wk, bk, Wv, bv, Wo, bo):
    from concourse.bass_utils import run_bass_kernel_spmd

    bf16 = ml_dtypes.bfloat16
    q = np.asarray(q, np.float32)
    k = np.asarray(k, np.float32)
    v = np.asarray(v, np.float32)

    xqT = [np.ascontiguousarray(q[b].T).astype(bf16) for b in range(2)]
    xkT = [np.ascontiguousarray(k[b].T).astype(bf16) for b in range(2)]
    xvT = [np.ascontiguousarray(v[b].T).astype(bf16) for b in range(2)]

    in_maps = []
    for c in range(NCORES):
        b, g = divmod(c, 4)
        cs = slice(CS * g, CS * (g + 1))
        in_maps.append({
            "xqT": xqT[b],
            "xkT": xkT[b],
            "xvT": xvT[b],
            "wq": np.ascontiguousarray(Wq[:, cs]).astype(bf16),
            "wk": np.ascontiguousarray(Wk[:, cs]).astype(bf16),
            "wv": np.ascontiguousarray(Wv[:, cs]).astype(bf16),
            "wo": np.ascontiguousarray(Wo[cs, :]).astype(np.float32),
            "bq": np.asarray(bq[cs], np.float32).reshape(CS, 1),
            "bk": np.asarray(bk[cs], np.float32).reshape(CS, 1),
            "bv": np.tile(np.asarray(bv[cs], np.float32), (128, 1)),
        })

    nc = _get_nc()
    res = run_bass_kernel_spmd(
        nc, in_maps, core_ids=list(range(NCORES)), **_cached.get("run_kwargs", {})
    )
    _cached["last_results"] = res

    out = np.zeros((2, S, D), np.float32)
    for c in range(NCORES):
        b = c // 4
        out[b] += res.results[c]["out"]
    out += np.asarray(bo, np.float32)
    return out



# revision 38
# speedup vs baseline: 1.3193x; 1.3193x over previous
"""MHA kernel for Trainium2, 8-way sharded (batch x head-group).

Reference: out = softmax((q@Wq+bq)(k@Wk+bk)^T / sqrt(64)) (v@Wv+bv) @ Wo + bo
Shapes: q,k,v [2, 2048, 768]; 12 heads x 64 dim.

Sharding (Megatron column-parallel): core c in 0..7 -> batch b = c//4,
head group g = c%4 (3 heads = channel slice 192g:192(g+1)). Each core
computes its heads' projections, attention, and partial out-proj
(Wo rows for its heads). Host sums the 4 partials per batch (+bo).

v2 restructure (pipeline overlap):
- DMA ring: xq/xk chunks interleaved (Q+K projections complete ~22us),
  then xv; weights ship host-prearranged [128, 6*192] for fat
  descriptors. Scores+exp run ahead of V readiness (pt pool depth 8).
- Loop order qb-outer/h-inner; out-projection emitted LAST (lowest
  priority) so the PE runs it only in attention stalls, except its
  deps gate it to (qb all-heads done). Its PSUM slots share the
  V-projection tag (temporally disjoint).
- R normalization tiles borrow the po PSUM slot (free between heads);
  po is copied to SBUF immediately on completion so the slot recycles.
- out-proj evacuation on DVE for qb0 (mid-attention) but on ACT for
  qb1 (tail, where ACT is idle).
- ohT and wo in bf16 (out-proj inputs).
- PSUM budget: scores 2x2 banks, po/R 1x2, outproj/V-proj 2x1 = 8.

Device layout notes:
- Host pre-transposes q/k/v to [768, 2048] (bf16) so projections can
  contract over the partition dim without on-device transposes.
- Scores are computed transposed: S^T[k, q] = Kh^T.T @ Qh^T, so exp
  output P^T[k, q] feeds PV as the moving operand with lhsT = [Vh | 1]
  (the appended ones column yields the softmax row-sums for free).
- Softmax skips max-subtraction: scores ~ N(0,1), no overflow in fp32.
- Normalization: r = 1/l broadcast across partitions via a K=1 matmul
  (ones[1,64].T @ r[1,q]), then DVE multiply during PSUM evacuation.
"""

import sys

if "/opt/trn_rl_repo" not in sys.path:
    sys.path.insert(0, "/opt/trn_rl_repo")

import numpy as np
import ml_dtypes

S = 2048
D = 768
DH = 64
HG = 3          # heads per core
CS = HG * DH    # 192 channel slice per core
NCORES = 8
ECH = D // 128  # 6 contraction chunks

_cached = {}


def _build_nc():
    import concourse.bass as bass
    from concourse import bacc, library_config
    import concourse.mybir as mybir
    import concourse.tile as tile

    f32 = mybir.dt.float32
    f32r = mybir.dt.float32r
    bf16 = mybir.dt.bfloat16

    nc = bacc.Bacc(None, target_bir_lowering=False)

    xq_d = nc.dram_tensor("xqT", [D, S], bf16, kind="ExternalInput")
    xk_d = nc.dram_tensor("xkT", [D, S], bf16, kind="ExternalInput")
    xv_d = nc.dram_tensor("xvT", [D, S], bf16, kind="ExternalInput")
    # weights host-prearranged to [128, ECH*CS] (chunk-major partitions)
    wq_d = nc.dram_tensor("wq", [128, ECH * CS], bf16, kind="ExternalInput")
    wk_d = nc.dram_tensor("wk", [128, ECH * CS], bf16, kind="ExternalInput")
    wv_d = nc.dram_tensor("wv", [128, ECH * CS], bf16, kind="ExternalInput")
    wo_d = nc.dram_tensor("wo", [CS, D], bf16, kind="ExternalInput")
    bq_d = nc.dram_tensor("bq", [CS, 1], f32, kind="ExternalInput")
    bk_d = nc.dram_tensor("bk", [CS, 1], f32, kind="ExternalInput")
    bv_d = nc.dram_tensor("bv", [128, CS], f32, kind="ExternalInput")
    out_d = nc.dram_tensor("out", [D, S], bf16, kind="ExternalOutput")  # out^T

    Exp = mybir.ActivationFunctionType.Exp
    PSUM = bass.MemorySpace.PSUM

    with tile.TileContext(nc) as tc:
        with (
            tc.tile_pool(name="cst", bufs=1) as cst,
            tc.tile_pool(name="big", bufs=1) as bigp,
            tc.tile_pool(name="pt", bufs=14) as ptp,
            tc.tile_pool(name="rr", bufs=2) as rrp,
            tc.tile_pool(name="osb", bufs=3) as osbp,
            tc.tile_pool(name="psA", bufs=2, space=PSUM) as psA,
            tc.tile_pool(name="psP", bufs=1, space=PSUM) as psP,
            tc.tile_pool(name="psO", bufs=2, space=PSUM) as psO,
        ):
            # gpsimd ucode library for partition_broadcast (attn lib)
            nc.gpsimd.load_library(library_config.attn)

            # ---- DMA emission order == arrival order on the sync ring ----
            # xv first: the V-projection drains in the otherwise-idle lead-in
            wv_sb = cst.tile([128, ECH, CS], bf16, tag="wv")
            nc.sync.dma_start(wv_sb[:], wv_d[:].rearrange("p (c n) -> p c n", c=ECH))
            wq_sb = cst.tile([128, ECH, CS], bf16, tag="wq")
            nc.sync.dma_start(wq_sb[:], wq_d[:].rearrange("p (c n) -> p c n", c=ECH))
            wk_sb = cst.tile([128, ECH, CS], bf16, tag="wk")
            nc.sync.dma_start(wk_sb[:], wk_d[:].rearrange("p (c n) -> p c n", c=ECH))
            bv_sb = cst.tile([128, CS], f32, tag="bv")
            nc.sync.dma_start(bv_sb[:], bv_d[:])

            xv_sb = bigp.tile([128, ECH, S], bf16, tag="xv")
            for e in range(ECH):
                nc.sync.dma_start(xv_sb[:, e, :], xv_d[e * 128:(e + 1) * 128, :])

            # interleave xq/xk chunks so Q and K projections finish together
            xq_sb = bigp.tile([128, ECH, S], bf16, tag="xq")
            xk_sb = bigp.tile([128, ECH, S], bf16, tag="xk")
            for e in range(ECH):
                nc.sync.dma_start(xq_sb[:, e, :], xq_d[e * 128:(e + 1) * 128, :])
                nc.sync.dma_start(xk_sb[:, e, :], xk_d[e * 128:(e + 1) * 128, :])

            bq_a = cst.tile([128, 1], f32, tag="bqa")
            nc.sync.dma_start(bq_a[:], bq_d[0:128, :])
            bq_b = cst.tile([DH, 1], f32, tag="bqb")
            nc.sync.dma_start(bq_b[:], bq_d[128:CS, :])
            bk_a = cst.tile([128, 1], f32, tag="bka")
            nc.sync.dma_start(bk_a[:], bk_d[0:128, :])
            bk_b = cst.tile([DH, 1], f32, tag="bkb")
            nc.sync.dma_start(bk_b[:], bk_d[128:CS, :])

            wo_a = cst.tile([128, D], bf16, tag="woa")
            nc.sync.dma_start(wo_a[:], wo_d[0:128, :])
            wo_b = cst.tile([DH, D], bf16, tag="wob")
            nc.sync.dma_start(wo_b[:], wo_d[128:CS, :])

            # ---- Q/K projections, e-outer so each chunk is consumed on ----
            # ---- arrival: 8 open accumulation groups span all 8 banks  ----
            # Q/K transposed per-head layout: heads 0,1 stacked [128, S]; head2 [64, S]
            qhT_a = bigp.tile([128, S], f32r, tag="qa")
            qhT_b = bigp.tile([DH, S], f32r, tag="qb")
            khT_a = bigp.tile([128, S], f32r, tag="ka")
            khT_b = bigp.tile([DH, S], f32r, tag="kb")

            def qk_proj(sc, e_outer):
                # one 1024-wide output block (sc) of both Q and K projections
                q0 = sc * 1024
                qm0 = psA.tile([128, 1024], f32, tag="sc", name=f"qm0_{sc}")
                km0 = psA.tile([128, 1024], f32, tag="sc", name=f"km0_{sc}")
                qm1 = psP.tile([DH, 1024], f32, tag="po", name=f"qm1_{sc}")
                units = [
                    (km0[:, 0:512], wk_sb, 0, 128, xk_sb, 0),
                    (km0[:, 512:1024], wk_sb, 0, 128, xk_sb, 512),
                    (qm0[:, 0:512], wq_sb, 0, 128, xq_sb, 0),
                    (qm0[:, 512:1024], wq_sb, 0, 128, xq_sb, 512),
                    (qm1[:, 0:512], wq_sb, 128, DH, xq_sb, 0),
                    (qm1[:, 512:1024], wq_sb, 128, DH, xq_sb, 512),
                ]
                if e_outer:
                    order = [(e, u) for e in range(ECH) for u in units]
                else:
                    order = [(e, u) for u in units for e in range(ECH)]
                for e, (out, w_sb, mc0, mw, x_sb, xo) in order:
                    nc.tensor.matmul(
                        out,
                        w_sb[:, e, mc0:mc0 + mw],
                        x_sb[:, e, q0 + xo:q0 + xo + 512],
                        start=(e == 0),
                        stop=(e == ECH - 1),
                    )
                nc.vector.tensor_scalar_add(khT_a[:, q0:q0 + 1024], km0[:], bk_a[:])
                nc.vector.tensor_scalar_add(qhT_a[:, q0:q0 + 1024], qm0[:], bq_a[:])
                nc.vector.tensor_scalar_add(qhT_b[:, q0:q0 + 1024], qm1[:], bq_b[:])

            # V projection first in the PE stream (xv arrives first); the
            # remaining sc1 / head-2 projection units are woven as filler
            # pieces inside the attention kc loops (psO slots, never psA).
            vh = bigp.tile([128, 16, HG, DH + 1], bf16, tag="vh")
            onecol = cst.tile([128, HG, 1], bf16, tag="onecol")
            nc.vector.memset(onecol[:], 1.0)

            _pp_state = {}

            def proj_piece(key, w_sb, x_sb, mc0, mw, bias, dest, off, ep):
                # one-third of a 512-wide projection unit (2 contraction
                # chunks) -- small enough to hide in per-kc ACT slack
                if ep == 0:
                    _pp_state[key] = psO.tile([mw, 512], f32, tag="op", name=key)
                ps = _pp_state[key]
                for e in (2 * ep, 2 * ep + 1):
                    nc.tensor.matmul(
                        ps[:],
                        w_sb[:, e, mc0:mc0 + mw],
                        x_sb[:, e, off:off + 512],
                        start=(e == 0),
                        stop=(e == ECH - 1),
                    )
                if ep == 2:
                    nc.vector.tensor_scalar_add(
                        dest[:, off:off + 512], ps[:], bias[:]
                    )

            def v_group(sb):
                ps = psO.tile([128, CS], f32, tag="op")
                for e in range(ECH):
                    nc.tensor.matmul(
                        ps[:],
                        xv_sb[:, e, sb * 128:(sb + 1) * 128],
                        wv_sb[:, e, :],
                        start=(e == 0),
                        stop=(e == ECH - 1),
                    )
                nc.vector.tensor_copy(vh[:, sb, :, DH:DH + 1], onecol[:])
                nc.vector.tensor_add(
                    vh[:, sb, :, 0:DH],
                    ps[:].rearrange("p (h d) -> p h d", h=HG),
                    bv_sb[:].rearrange("p (h d) -> p h d", h=HG),
                )

            for sb in range(16):
                v_group(sb)

            qk_proj(0, e_outer=True)   # paced by the xq/xk DMA arrival

            fillers = {}

            def _sched(qb, h, kc, fn):
                fillers.setdefault((qb, h, kc), []).append(fn)

            _pp_jobs = []
            for (t, w_sb, x_sb, mc0, mw, bias, dest, off) in (
                ("k", wk_sb, xk_sb, 0, 128, bk_a, khT_a, 1024),   # km0 sc1
                ("k", wk_sb, xk_sb, 0, 128, bk_a, khT_a, 1536),
                ("kb", wk_sb, xk_sb, 128, DH, bk_b, khT_b, 0),    # km1 sc0
                ("kb", wk_sb, xk_sb, 128, DH, bk_b, khT_b, 512),
                ("kb", wk_sb, xk_sb, 128, DH, bk_b, khT_b, 1024),  # km1 sc1
                ("kb", wk_sb, xk_sb, 128, DH, bk_b, khT_b, 1536),
                ("q", wq_sb, xq_sb, 0, 128, bq_a, qhT_a, 1024),   # qm0 sc1
                ("q", wq_sb, xq_sb, 0, 128, bq_a, qhT_a, 1536),
                ("qb", wq_sb, xq_sb, 128, DH, bq_b, qhT_b, 1024),  # qm1 sc1
                ("qb", wq_sb, xq_sb, 128, DH, bq_b, qhT_b, 1536),
            ):
                key = f"pp_{t}_{off}"
                for ep in range(3):
                    _pp_jobs.append(
                        lambda key=key, w_sb=w_sb, x_sb=x_sb, mc0=mc0, mw=mw,
                               bias=bias, dest=dest, off=off, ep=ep:
                        proj_piece(key, w_sb, x_sb, mc0, mw, bias, dest, off, ep)
                    )
            # 30 pieces. khT_a sc1 (first 6) must land by (0,0,kc7); khT_b
            # sc0 (next 6) by end of h1; the rest is read only in qb1.
            # Spacing ~every other kc keeps each piece inside the ACT slack.
            _slots = [(0, 0, kc) for kc in (1, 2, 3, 4, 5, 6)]      # km0 sc1
            _slots += [(0, 1, kc) for kc in range(0, 12, 2)]          # km1 sc0
            _slots += [(1, 0, kc) for kc in range(0, 12, 2)]          # km1 sc1
            _slots += [(0, 2, kc) for kc in range(0, 12, 2)]          # qm0 sc1
            _slots += [(0, 2, kc) for kc in (12, 14)]                 # qm1 sc1
            _slots += [(1, 0, kc) for kc in (12, 14)]
            _slots += [(1, 1, kc) for kc in (1, 3)]
            for job, slot in zip(_pp_jobs, _slots):
                _sched(*slot, job)

            # ---- attention + out-projection, pipelined per q-block ----
            # ohT stacked like qhT: heads 0,1 in [128, S]; head 2 in [64, S]
            ohT_a = bigp.tile([128, S], bf16, tag="oha", name="ohT_a")
            ohT_b = bigp.tile([DH, S], bf16, tag="ohb", name="ohT_b")

            def normalize(h, qb):
                # r = 1/l ; broadcast r across partitions on the (idle) Pool
                # engine; one DVE multiply writes the normalized ohT slice.
                q0 = qb * 1024
                poS, r_sb = pending.pop(0)
                R_sb = rrp.tile([DH, 1024], f32, tag="R")
                nc.gpsimd.partition_broadcast(R_sb[:], r_sb[:])
                dst = ohT_a[h * DH:(h + 1) * DH] if h < 2 else ohT_b[:]
                nc.vector.tensor_mul(
                    dst[:, q0:q0 + 1024], poS[0:DH, :], R_sb[:]
                )

            def normalize_last(po, h, qb):
                # latency-critical final head: skip the SBUF copy, read po
                # PSUM directly, and pipeline the chain in 256-quarters so
                # the tail out-proj can start after the first half.
                q0 = qb * 1024
                dst = ohT_a[h * DH:(h + 1) * DH] if h < 2 else ohT_b[:]
                for nh in range(2):
                    cs = slice(nh * 512, (nh + 1) * 512)
                    r_sb = rrp.tile([1, 512], f32, tag="rl", bufs=2)
                    nc.vector.reciprocal(r_sb[:], po[DH:DH + 1, cs])
                    R_sb = rrp.tile([DH, 512], f32, tag="Rl", bufs=2)
                    nc.gpsimd.partition_broadcast(R_sb[:], r_sb[:])
                    nc.vector.tensor_mul(
                        dst[:, q0 + nh * 512:q0 + (nh + 1) * 512],
                        po[0:DH, cs], R_sb[:],
                    )

            def out_proj_mm(ps, qblk, ocs):
                # psum[128, 512] = out^T[ocs*128:(ocs+1)*128, qblk*512:...]:
                # wo^T-chunks @ ohT-chunks (contraction over the stacked 192
                # head-channels, 128+64)
                qs = slice(qblk * 512, (qblk + 1) * 512)
                os_ = slice(ocs * 128, (ocs + 1) * 128)
                nc.tensor.matmul(ps[:], wo_a[:, os_], ohT_a[:, qs], start=True, stop=False)
                nc.tensor.matmul(ps[:], wo_b[:, os_], ohT_b[:, qs], start=False, stop=True)

            def out_proj(qblk, ocs, tail=False):
                ps = psO.tile([128, 512], f32, tag="op")
                out_proj_mm(ps, qblk, ocs)
                o_sb = osbp.tile([128, 512], bf16, tag="osb")
                nc.vector.tensor_copy(o_sb[:], ps[:])
                nc.sync.dma_start(
                    out_d[ocs * 128:(ocs + 1) * 128, qblk * 512:(qblk + 1) * 512], o_sb[:]
                )

            def out_proj_tail(ocs):
                # both q halves of the row block in one [128, 1024] psum tile
                # (4 MMs), one evacuation, one fat DMA. Tiles rotate over the
                # freed scores slots (psA x2) and the po slot (psP); evacs
                # alternate DVE/ACT (both idle in the tail).
                pool, tag = [(psA, "sc"), (psP, "po")][ocs % 2]
                ps = pool.tile([128, 1024], f32, tag=tag, name=f"opt{ocs}")
                for qblk in (2, 3):
                    out_proj_mm(ps[:, (qblk - 2) * 512:(qblk - 1) * 512], qblk, ocs)
                o_sb = osbp.tile([128, 1024], bf16, tag="osbt", bufs=3)
                if ocs % 2:
                    nc.scalar.copy(o_sb[:], ps[:])
                else:
                    nc.vector.tensor_copy(o_sb[:], ps[:])
                nc.sync.dma_start(out_d[ocs * 128:(ocs + 1) * 128, 1024:2048], o_sb[:])

            # qb0 out-proj groups woven into qb1's attention stream
            og = [(qblk, ocs) for qblk in (0, 1) for ocs in range(6)]
            for i, (qblk, ocs) in enumerate(og):
                h, kc = divmod(i, 6)
                fillers.setdefault((1, 1 + h, 2 * kc + 2), []).append(
                    lambda qblk=qblk, ocs=ocs: out_proj(qblk, ocs)
                )

            pending = []  # (poS, r_sb) of the head awaiting normalization
            prev = None   # (h, qb) of that head
            for qb in range(2):
                q0 = qb * 1024
                for h in range(HG):
                    if h < 2:
                        qh = qhT_a[h * DH:(h + 1) * DH, :]
                        kh = khT_a[h * DH:(h + 1) * DH, :]
                    else:
                        qh = qhT_b[:, :]
                        kh = khT_b[:, :]
                    po = psP.tile([DH + 1, 1024], f32, tag="po")
                    for kc in range(16):  # k chunks of 128
                        ps = psA.tile([128, 1024], f32, tag="sc")
                        for nh in range(2):
                            nc.tensor.matmul(
                                ps[:, nh * 512:(nh + 1) * 512],
                                kh[:, kc * 128:(kc + 1) * 128],
                                qh[:, q0 + nh * 512:q0 + (nh + 1) * 512],
                            )
                        pt = ptp.tile([128, 1024], bf16, tag="pt")
                        nc.scalar.activation(pt[:], ps[:], Exp, scale=0.125)
                        for nh in range(2):
                            nc.tensor.matmul(
                                po[:, nh * 512:(nh + 1) * 512],
                                vh[:, kc, h, :],
                                pt[:, nh * 512:(nh + 1) * 512],
                                start=(kc == 0),
                                stop=(kc == 15),
                            )
                        if kc == 2 and prev is not None:
                            normalize(*prev)
                            prev = None
                        for f in fillers.pop((qb, h, kc), ()):
                            f()
                    if qb == 1 and h == HG - 1:
                        normalize_last(po, h, qb)
                    else:
                        # evacuate po to SBUF immediately (frees the PSUM slot)
                        poS = rrp.tile([DH + 1, 1024], f32, tag="poS")
                        nc.vector.tensor_copy(poS[:], po[:])
                        r_sb = rrp.tile([1, 1024], f32, tag="r", bufs=2)
                        nc.vector.reciprocal(r_sb[:], poS[DH:DH + 1, :])
                        pending.append((poS, r_sb))
                        prev = (h, qb)

            # tail: out-proj for qb1's q rows
            for ocs in range(6):
                out_proj_tail(ocs)

    nc.compile()
    return nc


def _w_prearr(w):
    # [768, 192] -> [128, 6*192] chunk-major partition layout (bf16)
    return np.ascontiguousarray(
        np.asarray(w, np.float32).reshape(ECH, 128, CS).transpose(1, 0, 2).reshape(128, ECH * CS)
    ).astype(ml_dtypes.bfloat16)


def _get_nc():
    if "nc" not in _cached:
        _cached["nc"] = _build_nc()
    return _cached["nc"]


def kernel(q, k, v, Wq, bq, Wk, bk, Wv, bv, Wo, bo):
    from concourse.bass_utils import run_bass_kernel_spmd

    bf16 = ml_dtypes.bfloat16
    q = np.asarray(q, np.float32)
    k = np.asarray(k, np.float32)
    v = np.asarray(v, np.float32)

    xqT = [np.ascontiguousarray(q[b].T).astype(bf16) for b in range(2)]
    xkT = [np.ascontiguousarray(k[b].T).astype(bf16) for b in range(2)]
    xvT = [np.ascontiguousarray(v[b].T).astype(bf16) for b in range(2)]

    in_maps = []
    for c in range(NCORES):
        b, g = divmod(c, 4)
        cs = slice(CS * g, CS * (g + 1))
        in_maps.append({
            "xqT": xqT[b],
            "xkT": xkT[b],
            "xvT": xvT[b],
            "wq": _w_prearr(Wq[:, cs]),
            "wk": _w_prearr(Wk[:, cs]),
            "wv": _w_prearr(Wv[:, cs]),
            "wo": np.ascontiguousarray(Wo[cs, :]).astype(bf16),
            "bq": np.asarray(bq[cs], np.float32).reshape(CS, 1),
            "bk": np.asarray(bk[cs], np.float32).reshape(CS, 1),
            "bv": np.tile(np.asarray(bv[cs], np.float32), (128, 1)),
        })

    nc = _get_nc()
    res = run_bass_kernel_spmd(
        nc, in_maps, core_ids=list(range(NCORES)), **_cached.get("run_kwargs", {})
    )
    _cached["last_results"] = res

    out = np.zeros((2, S, D), np.float32)
    for c in range(NCORES):
        b = c // 4
        out[b] += np.asarray(res.results[c]["out"], np.float32).T
    out += np.asarray(bo, np.float32)
    return out


# revision 40
# speedup vs baseline: 1.3539x; 1.0262x over previous
"""MHA kernel for Trainium2, 8-way sharded (batch x head-group).

Reference: out = softmax((q@Wq+bq)(k@Wk+bk)^T / sqrt(64)) (v@Wv+bv) @ Wo + bo
Shapes: q,k,v [2, 2048, 768]; 12 heads x 64 dim.

Sharding (Megatron column-parallel): core c in 0..7 -> batch b = c//4,
head group g = c%4 (3 heads = channel slice 192g:192(g+1)). Each core
computes its heads' projections, attention, and partial out-proj
(Wo rows for its heads). Host sums the 4 partials per batch (+bo).

v2 restructure (pipeline overlap; cost-model-guided):
- DMA ring: wv/wq/wk, xv, then xq/xk interleaved, biases, wo. The
  V-projection drains in the otherwise-idle lead-in; Q/K projections
  consume chunks on arrival (e-outer, 6 open PSUM groups); the
  remaining sc1 / head-2 projection units are woven into the attention
  kc loops as 2-chunk pieces small enough to hide in per-kc ACT slack
  (each emitted before its reader: khT pieces are KEY positions, read
  from kc8 of every head; qhT pieces are q positions, read in qb1).
- The attention phase is ACT(exp)-bound: ~1.04us per [128,1024] tile,
  96 tiles. Everything else (projections, PV, out-proj, normalize)
  hides in PE/DVE/Pool slack around that stream.
- Normalization: r = 1/l on DVE, partition-broadcast on the idle
  GPSIMD engine (attn ucode library), one DVE multiply -> ohT (bf16).
- Out-projection is TRANSPOSED (out^T[oc, q], contraction over the
  stacked 192 head-channels as 128+64 chunks; host transposes back):
  24576 PE cycles instead of 36864. qb0 groups weave into qb1's
  attention; qb1 groups form the tail, [128,1024] psum tiles rotating
  over freed psA/psP slots, evacs alternating DVE/ACT, bf16 output
  DMA. The last head normalizes straight from PSUM in 512-halves.
- pt pool depth 14 decouples exp run-ahead from PV (which waits on
  vh early in head 0).
- PSUM budget: scores 2x2 banks, po/qm1 1x2, psO (V/pieces/outproj)
  2x1 = 8 banks.

Device layout notes:
- Host pre-transposes q/k/v to [768, 2048] (bf16) so projections can
  contract over the partition dim without on-device transposes.
- Scores are computed transposed: S^T[k, q] = Kh^T.T @ Qh^T, so exp
  output P^T[k, q] feeds PV as the moving operand with lhsT = [Vh | 1]
  (the appended ones column yields the softmax row-sums for free).
- Softmax skips max-subtraction: scores ~ N(0,1), no overflow in fp32.
- Normalization: r = 1/l broadcast across partitions via a K=1 matmul
  (ones[1,64].T @ r[1,q]), then DVE multiply during PSUM evacuation.
"""

import sys

if "/opt/trn_rl_repo" not in sys.path:
    sys.path.insert(0, "/opt/trn_rl_repo")

import numpy as np
import ml_dtypes

S = 2048
D = 768
DH = 64
HG = 3          # heads per core
CS = HG * DH    # 192 channel slice per core
NCORES = 8
ECH = D // 128  # 6 contraction chunks

_cached = {}


def _build_nc():
    import concourse.bass as bass
    from concourse import bacc, library_config
    import concourse.mybir as mybir
    import concourse.tile as tile

    f32 = mybir.dt.float32
    f32r = mybir.dt.float32r
    bf16 = mybir.dt.bfloat16

    nc = bacc.Bacc(None, target_bir_lowering=False)

    xq_d = nc.dram_tensor("xqT", [D, S], bf16, kind="ExternalInput")
    xk_d = nc.dram_tensor("xkT", [D, S], bf16, kind="ExternalInput")
    xv_d = nc.dram_tensor("xvT", [D, S], bf16, kind="ExternalInput")
    # weights host-prearranged to [128, ECH*CS] (chunk-major partitions)
    wq_d = nc.dram_tensor("wq", [128, ECH * CS], bf16, kind="ExternalInput")
    wk_d = nc.dram_tensor("wk", [128, ECH * CS], bf16, kind="ExternalInput")
    wv_d = nc.dram_tensor("wv", [128, ECH * CS], bf16, kind="ExternalInput")
    wo_d = nc.dram_tensor("wo", [CS, D], bf16, kind="ExternalInput")
    bq_d = nc.dram_tensor("bq", [CS, 1], f32, kind="ExternalInput")
    bk_d = nc.dram_tensor("bk", [CS, 1], f32, kind="ExternalInput")
    bv_d = nc.dram_tensor("bv", [128, CS], f32, kind="ExternalInput")
    out_d = nc.dram_tensor("out", [D, S], bf16, kind="ExternalOutput")  # out^T

    Exp = mybir.ActivationFunctionType.Exp
    PSUM = bass.MemorySpace.PSUM

    with tile.TileContext(nc) as tc:
        with (
            tc.tile_pool(name="cst", bufs=1) as cst,
            tc.tile_pool(name="big", bufs=1) as bigp,
            tc.tile_pool(name="pt", bufs=14) as ptp,
            tc.tile_pool(name="rr", bufs=2) as rrp,
            tc.tile_pool(name="osb", bufs=3) as osbp,
            tc.tile_pool(name="psA", bufs=2, space=PSUM) as psA,
            tc.tile_pool(name="psP", bufs=1, space=PSUM) as psP,
            tc.tile_pool(name="psO", bufs=2, space=PSUM) as psO,
        ):
            # gpsimd ucode library for partition_broadcast (attn lib)
            nc.gpsimd.load_library(library_config.attn)

            # ---- DMA emission order == arrival order on the sync ring ----
            # xv first: the V-projection drains in the otherwise-idle lead-in
            wv_sb = cst.tile([128, ECH, CS], bf16, tag="wv")
            nc.sync.dma_start(wv_sb[:], wv_d[:].rearrange("p (c n) -> p c n", c=ECH))
            wq_sb = cst.tile([128, ECH, CS], bf16, tag="wq")
            nc.sync.dma_start(wq_sb[:], wq_d[:].rearrange("p (c n) -> p c n", c=ECH))
            wk_sb = cst.tile([128, ECH, CS], bf16, tag="wk")
            nc.sync.dma_start(wk_sb[:], wk_d[:].rearrange("p (c n) -> p c n", c=ECH))
            bv_sb = cst.tile([128, CS], f32, tag="bv")
            nc.sync.dma_start(bv_sb[:], bv_d[:])

            xv_sb = bigp.tile([128, ECH, S], bf16, tag="xv")
            for e in range(ECH):
                nc.sync.dma_start(xv_sb[:, e, :], xv_d[e * 128:(e + 1) * 128, :])

            # interleave xq/xk chunks so Q and K projections finish together
            xq_sb = bigp.tile([128, ECH, S], bf16, tag="xq")
            xk_sb = bigp.tile([128, ECH, S], bf16, tag="xk")
            for e in range(ECH):
                nc.sync.dma_start(xq_sb[:, e, :], xq_d[e * 128:(e + 1) * 128, :])
                nc.sync.dma_start(xk_sb[:, e, :], xk_d[e * 128:(e + 1) * 128, :])

            bq_a = cst.tile([128, 1], f32, tag="bqa")
            nc.sync.dma_start(bq_a[:], bq_d[0:128, :])
            bq_b = cst.tile([DH, 1], f32, tag="bqb")
            nc.sync.dma_start(bq_b[:], bq_d[128:CS, :])
            bk_a = cst.tile([128, 1], f32, tag="bka")
            nc.sync.dma_start(bk_a[:], bk_d[0:128, :])
            bk_b = cst.tile([DH, 1], f32, tag="bkb")
            nc.sync.dma_start(bk_b[:], bk_d[128:CS, :])

            wo_a = cst.tile([128, D], bf16, tag="woa")
            nc.sync.dma_start(wo_a[:], wo_d[0:128, :])
            wo_b = cst.tile([DH, D], bf16, tag="wob")
            nc.sync.dma_start(wo_b[:], wo_d[128:CS, :])

            # ---- Q/K projections, e-outer so each chunk is consumed on ----
            # ---- arrival: 8 open accumulation groups span all 8 banks  ----
            # Q/K transposed per-head layout: heads 0,1 stacked [128, S]; head2 [64, S]
            qhT_a = bigp.tile([128, S], f32r, tag="qa")
            qhT_b = bigp.tile([DH, S], f32r, tag="qb")
            khT_a = bigp.tile([128, S], f32r, tag="ka")
            khT_b = bigp.tile([DH, S], f32r, tag="kb")

            def qk_proj(sc, e_outer):
                # one 1024-wide output block (sc) of both Q and K projections
                q0 = sc * 1024
                qm0 = psA.tile([128, 1024], f32, tag="sc", name=f"qm0_{sc}")
                km0 = psA.tile([128, 1024], f32, tag="sc", name=f"km0_{sc}")
                qm1 = psP.tile([DH, 1024], f32, tag="po", name=f"qm1_{sc}")
                units = [
                    (km0[:, 0:512], wk_sb, 0, 128, xk_sb, 0),
                    (km0[:, 512:1024], wk_sb, 0, 128, xk_sb, 512),
                    (qm0[:, 0:512], wq_sb, 0, 128, xq_sb, 0),
                    (qm0[:, 512:1024], wq_sb, 0, 128, xq_sb, 512),
                    (qm1[:, 0:512], wq_sb, 128, DH, xq_sb, 0),
                    (qm1[:, 512:1024], wq_sb, 128, DH, xq_sb, 512),
                ]
                if e_outer:
                    order = [(e, u) for e in range(ECH) for u in units]
                else:
                    order = [(e, u) for u in units for e in range(ECH)]
                for e, (out, w_sb, mc0, mw, x_sb, xo) in order:
                    nc.tensor.matmul(
                        out,
                        w_sb[:, e, mc0:mc0 + mw],
                        x_sb[:, e, q0 + xo:q0 + xo + 512],
                        start=(e == 0),
                        stop=(e == ECH - 1),
                    )
                nc.vector.tensor_scalar_add(khT_a[:, q0:q0 + 1024], km0[:], bk_a[:])
                nc.vector.tensor_scalar_add(qhT_a[:, q0:q0 + 1024], qm0[:], bq_a[:])
                nc.vector.tensor_scalar_add(qhT_b[:, q0:q0 + 1024], qm1[:], bq_b[:])

            # V projection first in the PE stream (xv arrives first); the
            # remaining sc1 / head-2 projection units are woven as filler
            # pieces inside the attention kc loops (psO slots, never psA).
            vh = bigp.tile([128, 16, HG, DH + 1], bf16, tag="vh")
            onecol = cst.tile([128, HG, 1], bf16, tag="onecol")
            nc.vector.memset(onecol[:], 1.0)

            _pp_state = {}

            def proj_piece(key, w_sb, x_sb, mc0, mw, bias, dest, off, ep):
                # one-third of a 512-wide projection unit (2 contraction
                # chunks) -- small enough to hide in per-kc ACT slack
                if ep == 0:
                    _pp_state[key] = psO.tile([mw, 512], f32, tag="op", name=key)
                ps = _pp_state[key]
                for e in (2 * ep, 2 * ep + 1):
                    nc.tensor.matmul(
                        ps[:],
                        w_sb[:, e, mc0:mc0 + mw],
                        x_sb[:, e, off:off + 512],
                        start=(e == 0),
                        stop=(e == ECH - 1),
                    )
                if ep == 2:
                    nc.vector.tensor_scalar_add(
                        dest[:, off:off + 512], ps[:], bias[:]
                    )

            def v_group(sb):
                ps = psO.tile([128, CS], f32, tag="op")
                for e in range(ECH):
                    nc.tensor.matmul(
                        ps[:],
                        xv_sb[:, e, sb * 128:(sb + 1) * 128],
                        wv_sb[:, e, :],
                        start=(e == 0),
                        stop=(e == ECH - 1),
                    )
                nc.vector.tensor_copy(vh[:, sb, :, DH:DH + 1], onecol[:])
                nc.vector.tensor_add(
                    vh[:, sb, :, 0:DH],
                    ps[:].rearrange("p (h d) -> p h d", h=HG),
                    bv_sb[:].rearrange("p (h d) -> p h d", h=HG),
                )

            for sb in range(16):
                v_group(sb)

            qk_proj(0, e_outer=True)   # paced by the xq/xk DMA arrival

            fillers = {}

            def _sched(qb, h, kc, fn):
                fillers.setdefault((qb, h, kc), []).append(fn)

            _pp_jobs = []
            for (t, w_sb, x_sb, mc0, mw, bias, dest, off) in (
                ("k", wk_sb, xk_sb, 0, 128, bk_a, khT_a, 1024),   # km0 sc1
                ("k", wk_sb, xk_sb, 0, 128, bk_a, khT_a, 1536),
                ("kb", wk_sb, xk_sb, 128, DH, bk_b, khT_b, 0),    # km1 sc0
                ("kb", wk_sb, xk_sb, 128, DH, bk_b, khT_b, 512),
                ("kb", wk_sb, xk_sb, 128, DH, bk_b, khT_b, 1024),  # km1 sc1
                ("kb", wk_sb, xk_sb, 128, DH, bk_b, khT_b, 1536),
                ("q", wq_sb, xq_sb, 0, 128, bq_a, qhT_a, 1024),   # qm0 sc1
                ("q", wq_sb, xq_sb, 0, 128, bq_a, qhT_a, 1536),
                ("qb", wq_sb, xq_sb, 128, DH, bq_b, qhT_b, 1024),  # qm1 sc1
                ("qb", wq_sb, xq_sb, 128, DH, bq_b, qhT_b, 1536),
            ):
                key = f"pp_{t}_{off}"
                for ep in range(3):
                    _pp_jobs.append(
                        lambda key=key, w_sb=w_sb, x_sb=x_sb, mc0=mc0, mw=mw,
                               bias=bias, dest=dest, off=off, ep=ep:
                        proj_piece(key, w_sb, x_sb, mc0, mw, bias, dest, off, ep)
                    )
            # 30 pieces. khT_a sc1 (first 6) must land by (0,0,kc7); khT_b
            # sc0 (next 6) by end of h1; the rest is read only in qb1.
            # Spacing ~every other kc keeps each piece inside the ACT slack.
            # deadlines: khT sc1 pieces are KEY positions 1024-2047, read
            # from kc8 of EVERY head; qhT sc1 pieces are q positions, read
            # only in qb1.
            _slots = [(0, 0, kc) for kc in (1, 2, 3, 4, 5, 6)]      # km0 sc1
            _slots += [(0, 1, kc) for kc in (0, 2, 4, 6, 8, 10)]     # km1 sc0
            _slots += [(0, 1, 3), (0, 1, 7), (0, 1, 11),
                       (0, 2, 1), (0, 2, 3), (0, 2, 5)]              # km1 sc1
            _slots += [(0, 2, kc) for kc in (0, 2, 4, 6, 8, 10)]     # qm0 sc1
            _slots += [(1, 0, kc) for kc in (1, 3, 5, 7, 9, 11)]     # qm1 sc1
            for job, slot in zip(_pp_jobs, _slots):
                _sched(*slot, job)

            # ---- attention + out-projection, pipelined per q-block ----
            # ohT stacked like qhT: heads 0,1 in [128, S]; head 2 in [64, S]
            ohT_a = bigp.tile([128, S], bf16, tag="oha", name="ohT_a")
            ohT_b = bigp.tile([DH, S], bf16, tag="ohb", name="ohT_b")

            def normalize(h, qb):
                # r = 1/l ; broadcast r across partitions on the (idle) Pool
                # engine; one DVE multiply writes the normalized ohT slice.
                q0 = qb * 1024
                poS, r_sb = pending.pop(0)
                R_sb = rrp.tile([DH, 1024], f32, tag="R")
                nc.gpsimd.partition_broadcast(R_sb[:], r_sb[:])
                dst = ohT_a[h * DH:(h + 1) * DH] if h < 2 else ohT_b[:]
                nc.vector.tensor_mul(
                    dst[:, q0:q0 + 1024], poS[0:DH, :], R_sb[:]
                )

            def normalize_last(po, h, qb):
                # latency-critical final head: skip the SBUF copy, read po
                # PSUM directly, and pipeline the chain in 256-quarters so
                # the tail out-proj can start after the first half.
                q0 = qb * 1024
                dst = ohT_a[h * DH:(h + 1) * DH] if h < 2 else ohT_b[:]
                for nh in range(2):
                    cs = slice(nh * 512, (nh + 1) * 512)
                    r_sb = rrp.tile([1, 512], f32, tag="rl", bufs=2)
                    nc.vector.reciprocal(r_sb[:], po[DH:DH + 1, cs])
                    R_sb = rrp.tile([DH, 512], f32, tag="Rl", bufs=2)
                    nc.gpsimd.partition_broadcast(R_sb[:], r_sb[:])
                    nc.vector.tensor_mul(
                        dst[:, q0 + nh * 512:q0 + (nh + 1) * 512],
                        po[0:DH, cs], R_sb[:],
                    )

            def out_proj_mm(ps, qblk, ocs):
                # psum[128, 512] = out^T[ocs*128:(ocs+1)*128, qblk*512:...]:
                # wo^T-chunks @ ohT-chunks (contraction over the stacked 192
                # head-channels, 128+64)
                qs = slice(qblk * 512, (qblk + 1) * 512)
                os_ = slice(ocs * 128, (ocs + 1) * 128)
                nc.tensor.matmul(ps[:], wo_a[:, os_], ohT_a[:, qs], start=True, stop=False)
                nc.tensor.matmul(ps[:], wo_b[:, os_], ohT_b[:, qs], start=False, stop=True)

            def out_proj(qblk, ocs, tail=False):
                ps = psO.tile([128, 512], f32, tag="op")
                out_proj_mm(ps, qblk, ocs)
                o_sb = osbp.tile([128, 512], bf16, tag="osb")
                nc.vector.tensor_copy(o_sb[:], ps[:])
                nc.sync.dma_start(
                    out_d[ocs * 128:(ocs + 1) * 128, qblk * 512:(qblk + 1) * 512], o_sb[:]
                )

            def out_proj_tail(ocs):
                # both q halves of the row block in one [128, 1024] psum tile
                # (4 MMs), one evacuation, one fat DMA. Tiles rotate over the
                # freed scores slots (psA x2) and the po slot (psP); evacs
                # alternate DVE/ACT (both idle in the tail).
                pool, tag = [(psA, "sc"), (psP, "po")][ocs % 2]
                ps = pool.tile([128, 1024], f32, tag=tag, name=f"opt{ocs}")
                for qblk in (2, 3):
                    out_proj_mm(ps[:, (qblk - 2) * 512:(qblk - 1) * 512], qblk, ocs)
                o_sb = osbp.tile([128, 1024], bf16, tag="osbt", bufs=3)
                if ocs % 2:
                    nc.scalar.copy(o_sb[:], ps[:])
                else:
                    nc.vector.tensor_copy(o_sb[:], ps[:])
                nc.sync.dma_start(out_d[ocs * 128:(ocs + 1) * 128, 1024:2048], o_sb[:])

            # qb0 out-proj groups woven into qb1's attention stream
            og = [(qblk, ocs) for qblk in (0, 1) for ocs in range(6)]
            for i, (qblk, ocs) in enumerate(og):
                h, kc = divmod(i, 6)
                fillers.setdefault((1, 1 + h, 2 * kc + 2), []).append(
                    lambda qblk=qblk, ocs=ocs: out_proj(qblk, ocs)
                )

            pending = []  # (poS, r_sb) of the head awaiting normalization
            prev = None   # (h, qb) of that head
            for qb in range(2):
                q0 = qb * 1024
                for h in range(HG):
                    if h < 2:
                        qh = qhT_a[h * DH:(h + 1) * DH, :]
                        kh = khT_a[h * DH:(h + 1) * DH, :]
                    else:
                        qh = qhT_b[:, :]
                        kh = khT_b[:, :]
                    po = psP.tile([DH + 1, 1024], f32, tag="po")
                    for kc in range(16):  # k chunks of 128
                        ps = psA.tile([128, 1024], f32, tag="sc")
                        for nh in range(2):
                            nc.tensor.matmul(
                                ps[:, nh * 512:(nh + 1) * 512],
                                kh[:, kc * 128:(kc + 1) * 128],
                                qh[:, q0 + nh * 512:q0 + (nh + 1) * 512],
                            )
                        pt = ptp.tile([128, 1024], bf16, tag="pt")
                        nc.scalar.activation(pt[:], ps[:], Exp, scale=0.125)
                        for nh in range(2):
                            nc.tensor.matmul(
                                po[:, nh * 512:(nh + 1) * 512],
                                vh[:, kc, h, :],
                                pt[:, nh * 512:(nh + 1) * 512],
                                start=(kc == 0),
                                stop=(kc == 15),
                            )
                        if kc == 2 and prev is not None:
                            normalize(*prev)
                            prev = None
                        for f in fillers.pop((qb, h, kc), ()):
                            f()
                    if qb == 1 and h == HG - 1:
                        normalize_last(po, h, qb)
                    else:
                        # evacuate po to SBUF immediately (frees the PSUM slot)
                        poS = rrp.tile([DH + 1, 1024], f32, tag="poS")
                        nc.vector.tensor_copy(poS[:], po[:])
                        r_sb = rrp.tile([1, 1024], f32, tag="r", bufs=2)
                        nc.vector.reciprocal(r_sb[:], poS[DH:DH + 1, :])
                        pending.append((poS, r_sb))
                        prev = (h, qb)

            # tail: out-proj for qb1's q rows
            for ocs in range(6):
                out_proj_tail(ocs)

    nc.compile()
    return nc


def _w_prearr(w):
    # [768, 192] -> [128, 6*192] chunk-major partition layout (bf16)
    return np.ascontiguousarray(
        np.asarray(w, np.float32).reshape(ECH, 128, CS).transpose(1, 0, 2).reshape(128, ECH * CS)
    ).astype(ml_dtypes.bfloat16)


def _get_nc():
    if "nc" not in _cached:
        _cached["nc"] = _build_nc()
    return _cached["nc"]


def kernel(q, k, v, Wq, bq, Wk, bk, Wv, bv, Wo, bo):
    from concourse.bass_utils import run_bass_kernel_spmd

    bf16 = ml_dtypes.bfloat16
    q = np.asarray(q, np.float32)
    k = np.asarray(k, np.float32)
    v = np.asarray(v, np.float32)

    xqT = [np.ascontiguousarray(q[b].T).astype(bf16) for b in range(2)]
    xkT = [np.ascontiguousarray(k[b].T).astype(bf16) for b in range(2)]
    xvT = [np.ascontiguousarray(v[b].T).astype(bf16) for b in range(2)]

    in_maps = []
    for c in range(NCORES):
        b, g = divmod(c, 4)
        cs = slice(CS * g, CS * (g + 1))
        in_maps.append({
            "xqT": xqT[b],
            "xkT": xkT[b],
            "xvT": xvT[b],
            "wq": _w_prearr(Wq[:, cs]),
            "wk": _w_prearr(Wk[:, cs]),
            "wv": _w_prearr(Wv[:, cs]),
            "wo": np.ascontiguousarray(Wo[cs, :]).astype(bf16),
            "bq": np.asarray(bq[cs], np.float32).reshape(CS, 1),
            "bk": np.asarray(bk[cs], np.float32).reshape(CS, 1),
            "bv": np.tile(np.asarray(bv[cs], np.float32), (128, 1)),
        })

    nc = _get_nc()
    res = run_bass_kernel_spmd(
        nc, in_maps, core_ids=list(range(NCORES)), **_cached.get("run_kwargs", {})
    )
    _cached["last_results"] = res

    out = np.zeros((2, S, D), np.float32)
    for c in range(NCORES):
        b = c // 4
        out[b] += np.asarray(res.results[c]["out"], np.float32).T
    out += np.asarray(bo, np.float32)
    return out


# revision 44
# speedup vs baseline: 1.3571x; 1.0023x over previous
"""MHA kernel for Trainium2, 8-way sharded (batch x head-group).

Reference: out = softmax((q@Wq+bq)(k@Wk+bk)^T / sqrt(64)) (v@Wv+bv) @ Wo + bo
Shapes: q,k,v [2, 2048, 768]; 12 heads x 64 dim.

Sharding (Megatron column-parallel): core c in 0..7 -> batch b = c//4,
head group g = c%4 (3 heads = channel slice 192g:192(g+1)). Each core
computes its heads' projections, attention, and partial out-proj
(Wo rows for its heads). Host sums the 4 partials per batch (+bo).

v2 restructure (pipeline overlap; cost-model-guided):
- DMA ring: wv/wq/wk, xv, then xq/xk interleaved, biases, wo. The
  V-projection drains in the otherwise-idle lead-in; Q/K projections
  consume chunks on arrival (e-outer, 6 open PSUM groups); the
  remaining sc1 / head-2 projection units are woven into the attention
  kc loops as 2-chunk pieces small enough to hide in per-kc ACT slack
  (each emitted before its reader: khT pieces are KEY positions, read
  from kc8 of every head; qhT pieces are q positions, read in qb1).
- The attention phase is ACT(exp)-bound: ~1.04us per [128,1024] tile,
  96 tiles. Everything else (projections, PV, out-proj, normalize)
  hides in PE/DVE/Pool slack around that stream.
- Normalization: r = 1/l on DVE, partition-broadcast on the idle
  GPSIMD engine (attn ucode library), one DVE multiply -> ohT (bf16).
- Out-projection is TRANSPOSED (out^T[oc, q], contraction over the
  stacked 192 head-channels as 128+64 chunks; host transposes back):
  24576 PE cycles instead of 36864. qb0 groups weave into qb1's
  attention; qb1 groups form the tail, [128,1024] psum tiles rotating
  over freed psA/psP slots, evacs alternating DVE/ACT, bf16 output
  DMA. The last head normalizes straight from PSUM in 512-halves.
- pt pool depth 14 decouples exp run-ahead from PV (which waits on
  vh early in head 0).
- PSUM budget: scores 2x2 banks, po/qm1 1x2, psO (V/pieces/outproj)
  2x1 = 8 banks.

Device layout notes:
- Host pre-transposes q/k/v to [768, 2048] (bf16) so projections can
  contract over the partition dim without on-device transposes.
- Scores are computed transposed: S^T[k, q] = Kh^T.T @ Qh^T, so exp
  output P^T[k, q] feeds PV as the moving operand with lhsT = [Vh | 1]
  (the appended ones column yields the softmax row-sums for free).
- Softmax skips max-subtraction: scores ~ N(0,1), no overflow in fp32.
- Normalization: r = 1/l broadcast across partitions via a K=1 matmul
  (ones[1,64].T @ r[1,q]), then DVE multiply during PSUM evacuation.
"""

import sys

if "/opt/trn_rl_repo" not in sys.path:
    sys.path.insert(0, "/opt/trn_rl_repo")

import numpy as np
import ml_dtypes

S = 2048
D = 768
DH = 64
HG = 3          # heads per core
CS = HG * DH    # 192 channel slice per core
NCORES = 8
ECH = D // 128  # 6 contraction chunks

_cached = {}


def _build_nc():
    import concourse.bass as bass
    from concourse import bacc, library_config
    import concourse.mybir as mybir
    import concourse.tile as tile

    f32 = mybir.dt.float32
    f32r = mybir.dt.float32r
    bf16 = mybir.dt.bfloat16

    nc = bacc.Bacc(None, target_bir_lowering=False)

    xq_d = nc.dram_tensor("xqT", [D, S], bf16, kind="ExternalInput")
    xk_d = nc.dram_tensor("xkT", [D, S], bf16, kind="ExternalInput")
    xv_d = nc.dram_tensor("xvT", [D, S], bf16, kind="ExternalInput")
    # weights host-prearranged to [128, ECH*CS] (chunk-major partitions)
    wq_d = nc.dram_tensor("wq", [128, ECH * CS], bf16, kind="ExternalInput")
    wk_d = nc.dram_tensor("wk", [128, ECH * CS], bf16, kind="ExternalInput")
    wv_d = nc.dram_tensor("wv", [128, ECH * CS], bf16, kind="ExternalInput")
    wo_d = nc.dram_tensor("wo", [CS, D], bf16, kind="ExternalInput")
    bq_d = nc.dram_tensor("bq", [CS, 1], f32, kind="ExternalInput")
    bk_d = nc.dram_tensor("bk", [CS, 1], f32, kind="ExternalInput")
    bv_d = nc.dram_tensor("bv", [128, CS], f32, kind="ExternalInput")
    out_d = nc.dram_tensor("out", [D, S], bf16, kind="ExternalOutput")  # out^T

    Exp = mybir.ActivationFunctionType.Exp
    PSUM = bass.MemorySpace.PSUM

    with tile.TileContext(nc) as tc:
        with (
            tc.tile_pool(name="cst", bufs=1) as cst,
            tc.tile_pool(name="big", bufs=1) as bigp,
            tc.tile_pool(name="pt", bufs=16) as ptp,
            tc.tile_pool(name="rr", bufs=2) as rrp,
            tc.tile_pool(name="osb", bufs=3) as osbp,
            tc.tile_pool(name="psA", bufs=2, space=PSUM) as psA,
            tc.tile_pool(name="psP", bufs=1, space=PSUM) as psP,
            tc.tile_pool(name="psO", bufs=2, space=PSUM) as psO,
        ):
            # gpsimd ucode library for partition_broadcast (attn lib)
            nc.gpsimd.load_library(library_config.attn)

            # ---- DMA emission order == arrival order on the sync ring ----
            # xv first: the V-projection drains in the otherwise-idle lead-in
            wv_sb = cst.tile([128, ECH, CS], bf16, tag="wv")
            nc.sync.dma_start(wv_sb[:], wv_d[:].rearrange("p (c n) -> p c n", c=ECH))
            wq_sb = cst.tile([128, ECH, CS], bf16, tag="wq")
            nc.sync.dma_start(wq_sb[:], wq_d[:].rearrange("p (c n) -> p c n", c=ECH))
            wk_sb = cst.tile([128, ECH, CS], bf16, tag="wk")
            nc.sync.dma_start(wk_sb[:], wk_d[:].rearrange("p (c n) -> p c n", c=ECH))
            bv_sb = cst.tile([128, CS], f32, tag="bv")
            nc.sync.dma_start(bv_sb[:], bv_d[:])

            xv_sb = bigp.tile([128, ECH, S], bf16, tag="xv")
            for e in range(ECH):
                nc.sync.dma_start(xv_sb[:, e, :], xv_d[e * 128:(e + 1) * 128, :])

            # interleave xq/xk chunks so Q and K projections finish together
            xq_sb = bigp.tile([128, ECH, S], bf16, tag="xq")
            xk_sb = bigp.tile([128, ECH, S], bf16, tag="xk")
            for e in range(ECH):
                nc.sync.dma_start(xq_sb[:, e, :], xq_d[e * 128:(e + 1) * 128, :])
                nc.sync.dma_start(xk_sb[:, e, :], xk_d[e * 128:(e + 1) * 128, :])

            bq_a = cst.tile([128, 1], f32, tag="bqa")
            nc.sync.dma_start(bq_a[:], bq_d[0:128, :])
            bq_b = cst.tile([DH, 1], f32, tag="bqb")
            nc.sync.dma_start(bq_b[:], bq_d[128:CS, :])
            bk_a = cst.tile([128, 1], f32, tag="bka")
            nc.sync.dma_start(bk_a[:], bk_d[0:128, :])
            bk_b = cst.tile([DH, 1], f32, tag="bkb")
            nc.sync.dma_start(bk_b[:], bk_d[128:CS, :])

            wo_a = cst.tile([128, D], bf16, tag="woa")
            nc.sync.dma_start(wo_a[:], wo_d[0:128, :])
            wo_b = cst.tile([DH, D], bf16, tag="wob")
            nc.sync.dma_start(wo_b[:], wo_d[128:CS, :])

            # ---- Q/K projections, e-outer so each chunk is consumed on ----
            # ---- arrival: 8 open accumulation groups span all 8 banks  ----
            # Q/K transposed per-head layout: heads 0,1 stacked [128, S]; head2 [64, S]
            qhT_a = bigp.tile([128, S], f32r, tag="qa")
            qhT_b = bigp.tile([DH, S], f32r, tag="qb")
            khT_a = bigp.tile([128, S], f32r, tag="ka")
            khT_b = bigp.tile([DH, S], f32r, tag="kb")

            def qk_proj(sc, e_outer):
                # one 1024-wide output block (sc) of both Q and K projections
                q0 = sc * 1024
                qm0 = psA.tile([128, 1024], f32, tag="sc", name=f"qm0_{sc}")
                km0 = psA.tile([128, 1024], f32, tag="sc", name=f"km0_{sc}")
                qm1 = psP.tile([DH, 1024], f32, tag="po", name=f"qm1_{sc}")
                units = [
                    (km0[:, 0:512], wk_sb, 0, 128, xk_sb, 0),
                    (km0[:, 512:1024], wk_sb, 0, 128, xk_sb, 512),
                    (qm0[:, 0:512], wq_sb, 0, 128, xq_sb, 0),
                    (qm0[:, 512:1024], wq_sb, 0, 128, xq_sb, 512),
                    (qm1[:, 0:512], wq_sb, 128, DH, xq_sb, 0),
                    (qm1[:, 512:1024], wq_sb, 128, DH, xq_sb, 512),
                ]
                if e_outer:
                    order = [(e, u) for e in range(ECH) for u in units]
                else:
                    order = [(e, u) for u in units for e in range(ECH)]
                for e, (out, w_sb, mc0, mw, x_sb, xo) in order:
                    nc.tensor.matmul(
                        out,
                        w_sb[:, e, mc0:mc0 + mw],
                        x_sb[:, e, q0 + xo:q0 + xo + 512],
                        start=(e == 0),
                        stop=(e == ECH - 1),
                    )
                # km0 evac on the (still idle) ACT engine so it runs in
                # parallel with qm0's DVE evac -- both gate scores(kc0)
                nc.scalar.add(khT_a[:, q0:q0 + 1024], km0[:], bk_a[:])
                nc.vector.tensor_scalar_add(qhT_a[:, q0:q0 + 1024], qm0[:], bq_a[:])
                nc.vector.tensor_scalar_add(qhT_b[:, q0:q0 + 1024], qm1[:], bq_b[:])

            # V projection first in the PE stream (xv arrives first); the
            # remaining sc1 / head-2 projection units are woven as filler
            # pieces inside the attention kc loops (psO slots, never psA).
            vh = bigp.tile([128, 16, HG, DH + 1], bf16, tag="vh")
            onecol = cst.tile([128, HG, 1], bf16, tag="onecol")
            nc.vector.memset(onecol[:], 1.0)

            _pp_state = {}

            def proj_piece(key, w_sb, x_sb, mc0, mw, bias, dest, off, ep):
                # one-third of a 512-wide projection unit (2 contraction
                # chunks) -- small enough to hide in per-kc ACT slack
                if ep == 0:
                    _pp_state[key] = psO.tile([mw, 512], f32, tag="op", name=key)
                ps = _pp_state[key]
                for e in (2 * ep, 2 * ep + 1):
                    nc.tensor.matmul(
                        ps[:],
                        w_sb[:, e, mc0:mc0 + mw],
                        x_sb[:, e, off:off + 512],
                        start=(e == 0),
                        stop=(e == ECH - 1),
                    )
                if ep == 2:
                    nc.vector.tensor_scalar_add(
                        dest[:, off:off + 512], ps[:], bias[:]
                    )

            def v_group(sb):
                ps = psO.tile([128, CS], f32, tag="op")
                for e in range(ECH):
                    nc.tensor.matmul(
                        ps[:],
                        xv_sb[:, e, sb * 128:(sb + 1) * 128],
                        wv_sb[:, e, :],
                        start=(e == 0),
                        stop=(e == ECH - 1),
                    )
                nc.vector.tensor_copy(vh[:, sb, :, DH:DH + 1], onecol[:])
                nc.vector.tensor_add(
                    vh[:, sb, :, 0:DH],
                    ps[:].rearrange("p (h d) -> p h d", h=HG),
                    bv_sb[:].rearrange("p (h d) -> p h d", h=HG),
                )

            for sb in range(16):
                v_group(sb)

            qk_proj(0, e_outer=True)   # paced by the xq/xk DMA arrival

            fillers = {}

            def _sched(qb, h, kc, fn):
                fillers.setdefault((qb, h, kc), []).append(fn)

            _pp_jobs = []
            for (t, w_sb, x_sb, mc0, mw, bias, dest, off) in (
                ("k", wk_sb, xk_sb, 0, 128, bk_a, khT_a, 1024),   # km0 sc1
                ("k", wk_sb, xk_sb, 0, 128, bk_a, khT_a, 1536),
                ("kb", wk_sb, xk_sb, 128, DH, bk_b, khT_b, 0),    # km1 sc0
                ("kb", wk_sb, xk_sb, 128, DH, bk_b, khT_b, 512),
                ("kb", wk_sb, xk_sb, 128, DH, bk_b, khT_b, 1024),  # km1 sc1
                ("kb", wk_sb, xk_sb, 128, DH, bk_b, khT_b, 1536),
                ("q", wq_sb, xq_sb, 0, 128, bq_a, qhT_a, 1024),   # qm0 sc1
                ("q", wq_sb, xq_sb, 0, 128, bq_a, qhT_a, 1536),
                ("qb", wq_sb, xq_sb, 128, DH, bq_b, qhT_b, 1024),  # qm1 sc1
                ("qb", wq_sb, xq_sb, 128, DH, bq_b, qhT_b, 1536),
            ):
                key = f"pp_{t}_{off}"
                for ep in range(3):
                    _pp_jobs.append(
                        lambda key=key, w_sb=w_sb, x_sb=x_sb, mc0=mc0, mw=mw,
                               bias=bias, dest=dest, off=off, ep=ep:
                        proj_piece(key, w_sb, x_sb, mc0, mw, bias, dest, off, ep)
                    )
            # 30 pieces. khT_a sc1 (first 6) must land by (0,0,kc7); khT_b
            # sc0 (next 6) by end of h1; the rest is read only in qb1.
            # Spacing ~every other kc keeps each piece inside the ACT slack.
            # deadlines: khT sc1 pieces are KEY positions 1024-2047, read
            # from kc8 of EVERY head; qhT sc1 pieces are q positions, read
            # only in qb1.
            _slots = [(0, 0, kc) for kc in (1, 2, 3, 4, 5, 6)]      # km0 sc1
            _slots += [(0, 1, kc) for kc in (0, 2, 4, 6, 8, 10)]     # km1 sc0
            _slots += [(0, 1, 3), (0, 1, 7), (0, 1, 11),
                       (0, 2, 1), (0, 2, 3), (0, 2, 5)]              # km1 sc1
            _slots += [(0, 2, kc) for kc in (0, 2, 4, 6, 8, 10)]     # qm0 sc1
            _slots += [(1, 0, kc) for kc in (1, 3, 5, 7, 9, 11)]     # qm1 sc1
            for job, slot in zip(_pp_jobs, _slots):
                _sched(*slot, job)

            # ---- attention + out-projection, pipelined per q-block ----
            # ohT stacked like qhT: heads 0,1 in [128, S]; head 2 in [64, S]
            ohT_a = bigp.tile([128, S], bf16, tag="oha", name="ohT_a")
            ohT_b = bigp.tile([DH, S], bf16, tag="ohb", name="ohT_b")

            def normalize(h, qb):
                # r = 1/l ; broadcast r across partitions on the (idle) Pool
                # engine; one DVE multiply writes the normalized ohT slice.
                q0 = qb * 1024
                poS, r_sb = pending.pop(0)
                R_sb = rrp.tile([DH, 1024], f32, tag="R")
                nc.gpsimd.partition_broadcast(R_sb[:], r_sb[:])
                dst = ohT_a[h * DH:(h + 1) * DH] if h < 2 else ohT_b[:]
                nc.vector.tensor_mul(
                    dst[:, q0:q0 + 1024], poS[0:DH, :], R_sb[:]
                )

            def normalize_last(po, h, qb):
                # latency-critical final head: skip the SBUF copy, read po
                # PSUM directly, and pipeline the chain in 256-quarters so
                # the tail out-proj can start after the first half.
                q0 = qb * 1024
                dst = ohT_a[h * DH:(h + 1) * DH] if h < 2 else ohT_b[:]
                for nh in range(2):
                    cs = slice(nh * 512, (nh + 1) * 512)
                    r_sb = rrp.tile([1, 512], f32, tag="rl", bufs=2)
                    nc.vector.reciprocal(r_sb[:], po[DH:DH + 1, cs])
                    R_sb = rrp.tile([DH, 512], f32, tag="Rl", bufs=2)
                    nc.gpsimd.partition_broadcast(R_sb[:], r_sb[:])
                    nc.vector.tensor_mul(
                        dst[:, q0 + nh * 512:q0 + (nh + 1) * 512],
                        po[0:DH, cs], R_sb[:],
                    )

            def out_proj_mm(ps, qblk, ocs):
                # psum[128, 512] = out^T[ocs*128:(ocs+1)*128, qblk*512:...]:
                # wo^T-chunks @ ohT-chunks (contraction over the stacked 192
                # head-channels, 128+64)
                qs = slice(qblk * 512, (qblk + 1) * 512)
                os_ = slice(ocs * 128, (ocs + 1) * 128)
                nc.tensor.matmul(ps[:], wo_a[:, os_], ohT_a[:, qs], start=True, stop=False)
                nc.tensor.matmul(ps[:], wo_b[:, os_], ohT_b[:, qs], start=False, stop=True)

            def out_proj(qblk, ocs, tail=False):
                ps = psO.tile([128, 512], f32, tag="op")
                out_proj_mm(ps, qblk, ocs)
                o_sb = osbp.tile([128, 512], bf16, tag="osb")
                nc.vector.tensor_copy(o_sb[:], ps[:])
                nc.sync.dma_start(
                    out_d[ocs * 128:(ocs + 1) * 128, qblk * 512:(qblk + 1) * 512], o_sb[:]
                )

            def out_proj_tail(ocs):
                # both q halves of the row block in one [128, 1024] psum tile
                # (4 MMs), one evacuation, one fat DMA. Tiles rotate over the
                # freed scores slots (psA x2) and the po slot (psP); evacs
                # alternate DVE/ACT (both idle in the tail).
                pool, tag = [(psA, "sc"), (psP, "po")][ocs % 2]
                ps = pool.tile([128, 1024], f32, tag=tag, name=f"opt{ocs}")
                for qblk in (2, 3):
                    out_proj_mm(ps[:, (qblk - 2) * 512:(qblk - 1) * 512], qblk, ocs)
                o_sb = osbp.tile([128, 1024], bf16, tag="osbt", bufs=3)
                if ocs % 2:
                    nc.scalar.copy(o_sb[:], ps[:])
                else:
                    nc.vector.tensor_copy(o_sb[:], ps[:])
                nc.sync.dma_start(out_d[ocs * 128:(ocs + 1) * 128, 1024:2048], o_sb[:])

            # qb0 out-proj groups woven into qb1's attention stream
            og = [(qblk, ocs) for qblk in (0, 1) for ocs in range(6)]
            for i, (qblk, ocs) in enumerate(og):
                h, kc = divmod(i, 6)
                fillers.setdefault((1, 1 + h, 2 * kc + 2), []).append(
                    lambda qblk=qblk, ocs=ocs: out_proj(qblk, ocs)
                )

            pending = []  # (poS, r_sb) of the head awaiting normalization
            prev = None   # (h, qb) of that head
            for qb in range(2):
                q0 = qb * 1024
                for h in range(HG):
                    if h < 2:
                        qh = qhT_a[h * DH:(h + 1) * DH, :]
                        kh = khT_a[h * DH:(h + 1) * DH, :]
                    else:
                        qh = qhT_b[:, :]
                        kh = khT_b[:, :]
                    po = psP.tile([DH + 1, 1024], f32, tag="po")
                    for kc in range(16):  # k chunks of 128
                        ps = psA.tile([128, 1024], f32, tag="sc")
                        for nh in range(2):
                            nc.tensor.matmul(
                                ps[:, nh * 512:(nh + 1) * 512],
                                kh[:, kc * 128:(kc + 1) * 128],
                                qh[:, q0 + nh * 512:q0 + (nh + 1) * 512],
                            )
                        pt = ptp.tile([128, 1024], bf16, tag="pt")
                        nc.scalar.activation(pt[:], ps[:], Exp, scale=0.125)
                        for nh in range(2):
                            nc.tensor.matmul(
                                po[:, nh * 512:(nh + 1) * 512],
                                vh[:, kc, h, :],
                                pt[:, nh * 512:(nh + 1) * 512],
                                start=(kc == 0),
                                stop=(kc == 15),
                            )
                        if kc == 2 and prev is not None:
                            normalize(*prev)
                            prev = None
                        for f in fillers.pop((qb, h, kc), ()):
                            f()
                    if qb == 1 and h == HG - 1:
                        normalize_last(po, h, qb)
                    else:
                        # evacuate po to SBUF immediately (frees the PSUM slot)
                        poS = rrp.tile([DH + 1, 1024], f32, tag="poS")
                        nc.vector.tensor_copy(poS[:], po[:])
                        r_sb = rrp.tile([1, 1024], f32, tag="r", bufs=2)
                        nc.vector.reciprocal(r_sb[:], poS[DH:DH + 1, :])
                        pending.append((poS, r_sb))
                        prev = (h, qb)

            # tail: out-proj for qb1's q rows
            for ocs in range(6):
                out_proj_tail(ocs)

    nc.compile()
    return nc


def _w_prearr(w):
    # [768, 192] -> [128, 6*192] chunk-major partition layout (bf16)
    return np.ascontiguousarray(
        np.asarray(w, np.float32).reshape(ECH, 128, CS).transpose(1, 0, 2).reshape(128, ECH * CS)
    ).astype(ml_dtypes.bfloat16)


def _get_nc():
    if "nc" not in _cached:
        _cached["nc"] = _build_nc()
    return _cached["nc"]


def kernel(q, k, v, Wq, bq, Wk, bk, Wv, bv, Wo, bo):
    from concourse.bass_utils import run_bass_kernel_spmd

    bf16 = ml_dtypes.bfloat16
    q = np.asarray(q, np.float32)
    k = np.asarray(k, np.float32)
    v = np.asarray(v, np.float32)

    xqT = [np.ascontiguousarray(q[b].T).astype(bf16) for b in range(2)]
    xkT = [np.ascontiguousarray(k[b].T).astype(bf16) for b in range(2)]
    xvT = [np.ascontiguousarray(v[b].T).astype(bf16) for b in range(2)]

    in_maps = []
    for c in range(NCORES):
        b, g = divmod(c, 4)
        cs = slice(CS * g, CS * (g + 1))
        in_maps.append({
            "xqT": xqT[b],
            "xkT": xkT[b],
            "xvT": xvT[b],
            "wq": _w_prearr(Wq[:, cs]),
            "wk": _w_prearr(Wk[:, cs]),
            "wv": _w_prearr(Wv[:, cs]),
            "wo": np.ascontiguousarray(Wo[cs, :]).astype(bf16),
            "bq": np.asarray(bq[cs], np.float32).reshape(CS, 1),
            "bk": np.asarray(bk[cs], np.float32).reshape(CS, 1),
            "bv": np.tile(np.asarray(bv[cs], np.float32), (128, 1)),
        })

    nc = _get_nc()
    res = run_bass_kernel_spmd(
        nc, in_maps, core_ids=list(range(NCORES)), **_cached.get("run_kwargs", {})
    )
    _cached["last_results"] = res

    out = np.zeros((2, S, D), np.float32)
    for c in range(NCORES):
        b = c // 4
        out[b] += np.asarray(res.results[c]["out"], np.float32).T
    out += np.asarray(bo, np.float32)
    return out
